# revision 26
# baseline (speedup 1.0000x reference)
"""LoRA-with-routing kernel for Trainium2 (8 NeuronCores, SPMD).

out[b] = base[b] + (x[b] @ lora_A[idx[b]]) @ lora_B[idx[b]] * s[idx[b]]

Sharding: data-parallel over batch (B=8 rows, one per core). The adapter
gather (routing) happens host-side while sharding: each core receives its
batch row plus that row's adapter weights (scale folded into B, cast bf16).

The kernel is HBM-bandwidth bound (~358 GB/s per core), so all HBM traffic
is 16-bit: x is pre-transposed/pre-swizzled host-side to [P, NG, DC, TG]
bf16 so each per-group load is one fully-contiguous 4 MiB DMA; base is
pre-cast to bf16; the output is stored bf16 and upcast host-side after the
gather. Per-core traffic: 16 (x) + 16 (base) + 16 (out) = 48 MiB.

Device pipeline per core (T=2048, D=4096, R=64), per 512-token group:
  1. load x group tile [128 p, 32 c, 512 t] bf16 (one 4 MiB DMA, gpsimd)
  2. GEMM1 (PE): interT[64 r, 512 t] += A_c.T @ x_c  (accum 32 d-chunks)
  3. DVE evac interT -> bf16 SBUF
  4. per 128-token subtile: load base bf16, GEMM2 y[128,512] = interT.T @ B,
     add into base (any engine), store bf16
"""

import sys

for _p in ("/opt/trn_rl_repo", "/root/.axon_site/_ro/trn_rl_repo"):
    if _p not in sys.path:
        sys.path.append(_p)

import numpy as np
import ml_dtypes

import concourse.bass as bass
import concourse.bacc as bacc
import concourse.mybir as mybir
from concourse import tile

B, T, D, R = 8, 2048, 4096, 64
P = 128          # partitions
DC = D // P      # 32 d-chunks (contraction)
DC2 = DC // 2    # 16 double-chunks (DoubleRow matmul: 2 k-rows/partition)
# Uniform fine groups keep all DMA streams (x/base/out) flowing together;
# long groups starve the store stream during GEMM1 phases.
GROUPS = [256] * 8
OCH = 512        # output free chunk (one PSUM bank of f32)
OC = D // OCH    # 8 o-chunks

F32 = mybir.dt.float32
BF16 = mybir.dt.bfloat16
# e4m3 for x and A enables the PE DoubleRow perf mode (2 contraction rows
# per cycle, halving GEMM1 time). A is scaled x64 on the host (its values
# are bounded by 1/64 = e4m3's min normal); 1/64 is folded into B.
FP8 = mybir.dt.float8e4


def build_program():
    nc = bacc.Bacc("TRN2", target_bir_lowering=False, debug=False, num_devices=B)
    # x pre-packed host-side, group-major then d-chunk-major per partition:
    # for group (t0, tg), columns [t0*DC : (t0+tg)*DC] hold [DC, tg] blocks
    # with xh[p, t0*DC + c*tg + t] = x[t0+t, c*128+p]
    xh = nc.dram_tensor("xh", [P, DC * T], FP8, kind="ExternalInput").ap()
    base = nc.dram_tensor("base", [T, D], BF16, kind="ExternalInput").ap()
    # A pre-swizzled host-side: a_w[p, c2, i, r] = 64*A[c2*256+i*128+p, r]
    a_w = nc.dram_tensor("a_w", [P, DC2, 2, R], FP8, kind="ExternalInput").ap()
    b_w = nc.dram_tensor("b_w", [R, D], BF16, kind="ExternalInput").ap()
    out = nc.dram_tensor("out", [T, D], BF16, kind="ExternalOutput").ap()

    with tile.TileContext(nc) as tc:
        _body(tc, xh, base, a_w, b_w, out)
    nc.compile()
    return nc


def _body(tc, xh, base, a_w, b_w, out):
    nc = tc.nc
    with (
        tc.tile_pool(name="const", bufs=1) as cpool,
        tc.tile_pool(name="xc", bufs=6) as xc_pool,
        tc.tile_pool(name="bs", bufs=12) as bs_pool,
        tc.tile_pool(name="ye", bufs=4) as ye_pool,
        tc.tile_pool(name="it", bufs=2) as it_pool,
        tc.tile_pool(name="ps1", bufs=2, space="PSUM") as ps1,
        tc.tile_pool(name="ps2", bufs=6, space="PSUM") as ps2,
    ):
        # Adapter weights, loaded once (contraction dim on partitions), on
        # the scalar engine's ring so the sync ring starts x/base at t=0.
        a_sb = cpool.tile([P, DC2, 2, R], FP8)
        nc.scalar.dma_start(a_sb[:], a_w[:])
        b_sb = cpool.tile([R, D], BF16)
        nc.scalar.dma_start(b_sb[:], b_w[:])

        t0 = 0
        for gi, tg in enumerate(GROUPS):
            # One contiguous load for the whole group's x (fp8).
            xc = xc_pool.tile([P, DC2, 2, tg], FP8)
            nc.sync.dma_start(xc[:], xh[:, t0 * DC : (t0 + tg) * DC])

            # GEMM1: interT[r, t] = 64 * sum_c A_c.T @ x_c, accumulated in
            # PSUM. DoubleRow: each partition carries 2 contraction rows
            # (d = c2*256 + i*128 + p), so 16 matmuls at 2 rows/cycle.
            it_ps = ps1.tile([R, tg], F32)
            for c2 in range(DC2):
                nc.tensor.matmul(
                    it_ps[:],
                    a_sb[:, c2, :, :],
                    xc[:, c2, :, :],
                    start=(c2 == 0),
                    stop=(c2 == DC2 - 1),
                    perf_mode=mybir.MatmulPerfMode.DoubleRow,
                )

            # evacuate to bf16 (GEMM2 stationary operand)
            it_sb = it_pool.tile([R, tg], BF16)
            nc.vector.tensor_copy(it_sb[:], it_ps[:])

            # The last group runs all adds on DVE (lowest-latency path) and
            # stores chunk-by-chunk to drain the kernel fast.
            drain = gi == len(GROUPS) - 1
            for sub in range(tg // P):
                tt = t0 + sub * P
                bs = bs_pool.tile([P, D], BF16)
                nc.sync.dma_start(bs[:], base[tt : tt + P, :])
                for o in range(OC):
                    y_ps = ps2.tile([P, OCH], F32)
                    nc.tensor.matmul(
                        y_ps[:],
                        it_sb[:, sub * P : (sub + 1) * P],
                        b_sb[:, o * OCH : (o + 1) * OCH],
                        start=True,
                        stop=True,
                    )
                    dst = bs[:, o * OCH : (o + 1) * OCH]
                    # Spread the PSUM-evac + base-add work across engines so
                    # PSUM frees faster than PE produces it. GpSimd can't
                    # read PSUM, so odd chunks go PSUM -> ACT copy -> SBUF ->
                    # GpSimd add.
                    if o % 2 == 0 or drain:
                        nc.vector.tensor_add(dst, dst, y_ps[:])
                    else:
                        ye = ye_pool.tile([P, OCH], BF16)
                        nc.scalar.activation(
                            ye[:], y_ps[:], mybir.ActivationFunctionType.Copy
                        )
                        nc.gpsimd.tensor_add(dst, dst, ye[:])
                    if drain and o % 2 == 1:
                        # store pairs of o-chunks as soon as their adds land,
                        # alternating rings to shorten the final flush
                        eng = nc.sync if o % 4 == 1 else nc.scalar
                        eng.dma_start(
                            out[tt : tt + P, (o - 1) * OCH : (o + 1) * OCH],
                            bs[:, (o - 1) * OCH : (o + 1) * OCH],
                        )
                if not drain:
                    nc.scalar.dma_start(out[tt : tt + P, :], bs[:])
            t0 += tg


def shard_inputs(x, base_output, adapter_indices, lora_A, lora_B, lora_scaling):
    idx = np.asarray(adapter_indices).astype(np.int64)
    a_b = np.asarray(lora_A, dtype=np.float32)[idx]        # [B, D, R]
    b_b = np.asarray(lora_B, dtype=np.float32)[idx]        # [B, R, D]
    s_b = np.asarray(lora_scaling, dtype=np.float32)[idx]  # [B]
    # A is scaled x64 into e4m3 normal range; the 1/64 is folded into B.
    b_scaled = (b_b * (s_b[:, None, None] / 64.0)).astype(ml_dtypes.bfloat16)
    # a_w[p, c2, i, r] = 64*A[c2*256+i*128+p, r]
    a_sw = (
        (64.0 * a_b)
        .reshape(B, DC2, 2, P, R)
        .transpose(0, 3, 1, 2, 4)
        .astype(ml_dtypes.float8_e4m3)
    )
    xs = np.asarray(x, dtype=np.float32)
    bs = np.asarray(base_output, dtype=np.float32).astype(ml_dtypes.bfloat16)
    maps = []
    for b in range(B):
        # group-major packing: per group (t0, tg) a [P, DC2, 2, tg] block,
        # xh[p, t0*DC + c2*2*tg + i*tg + t] = x[b, t0+t, c2*256+i*128+p]
        blocks = []
        t0 = 0
        xtb = xs[b].T.reshape(DC2, 2, P, T)  # [c2, i, p, t]
        for tg in GROUPS:
            blocks.append(
                xtb[:, :, :, t0 : t0 + tg].transpose(2, 0, 1, 3).reshape(P, DC * tg)
            )
            t0 += tg
        xt = np.concatenate(blocks, axis=1)
        maps.append(
            {
                "xh": np.ascontiguousarray(xt.astype(ml_dtypes.float8_e4m3)),
                "base": np.ascontiguousarray(bs[b]),
                "a_w": np.ascontiguousarray(a_sw[b]),
                "b_w": np.ascontiguousarray(b_scaled[b]),
            }
        )
    return maps


def run(inputs: dict, trace: bool = False, **kwargs):
    """Build + run on 8 cores. Returns (output [B,T,D] f32, BassKernelResults)."""
    from concourse.bass_utils import run_bass_kernel_spmd

    nc = build_program()
    in_maps = shard_inputs(**inputs)
    res = run_bass_kernel_spmd(
        nc, in_maps, core_ids=list(range(B)), trace=trace, **kwargs
    )
    out = np.stack(
        [np.asarray(res.results[b]["out"]).astype(np.float32) for b in range(B)],
        axis=0,
    )
    return out, res


def kernel(x, base_output, adapter_indices, lora_A, lora_B, lora_scaling):
    out, _ = run(
        dict(
            x=x,
            base_output=base_output,
            adapter_indices=adapter_indices,
            lora_A=lora_A,
            lora_B=lora_B,
            lora_scaling=lora_scaling,
        )
    )
    return out


# revision 27
# speedup vs baseline: 1.0301x; 1.0301x over previous
"""LoRA-with-routing kernel for Trainium2 (8 NeuronCores, SPMD).

out[b] = base[b] + (x[b] @ lora_A[idx[b]]) @ lora_B[idx[b]] * s[idx[b]]

Sharding: data-parallel over batch (B=8 rows, one per core). The adapter
gather (routing) happens host-side while sharding: each core receives its
batch row plus that row's adapter weights (scale folded into B, cast bf16).

The kernel is HBM-bandwidth bound (~358 GB/s per core), so all HBM traffic
is 16-bit: x is pre-transposed/pre-swizzled host-side to [P, NG, DC, TG]
bf16 so each per-group load is one fully-contiguous 4 MiB DMA; base is
pre-cast to bf16; the output is stored bf16 and upcast host-side after the
gather. Per-core traffic: 16 (x) + 16 (base) + 16 (out) = 48 MiB.

Device pipeline per core (T=2048, D=4096, R=64), per 512-token group:
  1. load x group tile [128 p, 32 c, 512 t] bf16 (one 4 MiB DMA, gpsimd)
  2. GEMM1 (PE): interT[64 r, 512 t] += A_c.T @ x_c  (accum 32 d-chunks)
  3. DVE evac interT -> bf16 SBUF
  4. per 128-token subtile: load base bf16, GEMM2 y[128,512] = interT.T @ B,
     add into base (any engine), store bf16
"""

import sys

for _p in ("/opt/trn_rl_repo", "/root/.axon_site/_ro/trn_rl_repo"):
    if _p not in sys.path:
        sys.path.append(_p)

import numpy as np
import ml_dtypes

import concourse.bass as bass
import concourse.bacc as bacc
import concourse.mybir as mybir
from concourse import tile

B, T, D, R = 8, 2048, 4096, 64
P = 128          # partitions
DC = D // P      # 32 d-chunks (contraction)
DC2 = DC // 2    # 16 double-chunks (DoubleRow matmul: 2 k-rows/partition)
# Uniform fine groups keep all DMA streams (x/base/out) flowing together;
# long groups starve the store stream during GEMM1 phases.
GROUPS = [256] * 8
OCH = 512        # output free chunk (one PSUM bank of f32)
OC = D // OCH    # 8 o-chunks

F32 = mybir.dt.float32
BF16 = mybir.dt.bfloat16
# e4m3 for x and A enables the PE DoubleRow perf mode (2 contraction rows
# per cycle, halving GEMM1 time). A is scaled x64 on the host (its values
# are bounded by 1/64 = e4m3's min normal); 1/64 is folded into B.
FP8 = mybir.dt.float8e4


def build_program():
    nc = bacc.Bacc("TRN2", target_bir_lowering=False, debug=False, num_devices=B)
    # x pre-packed host-side, group-major then d-chunk-major per partition:
    # for group (t0, tg), columns [t0*DC : (t0+tg)*DC] hold [DC, tg] blocks
    # with xh[p, t0*DC + c*tg + t] = x[t0+t, c*128+p]
    xh = nc.dram_tensor("xh", [P, DC * T], FP8, kind="ExternalInput").ap()
    base = nc.dram_tensor("base", [T, D], BF16, kind="ExternalInput").ap()
    # A pre-swizzled host-side: a_w[p, c2, i, r] = 64*A[c2*256+i*128+p, r]
    a_w = nc.dram_tensor("a_w", [P, DC2, 2, R], FP8, kind="ExternalInput").ap()
    b_w = nc.dram_tensor("b_w", [R, D], BF16, kind="ExternalInput").ap()
    out = nc.dram_tensor("out", [T, D], BF16, kind="ExternalOutput").ap()

    with tile.TileContext(nc) as tc:
        _body(tc, xh, base, a_w, b_w, out)
    nc.compile()
    return nc


def _body(tc, xh, base, a_w, b_w, out):
    nc = tc.nc
    with (
        tc.tile_pool(name="const", bufs=1) as cpool,
        tc.tile_pool(name="xc", bufs=4) as xc_pool,
        tc.tile_pool(name="bs", bufs=8) as bs_pool,
        tc.tile_pool(name="ye", bufs=4) as ye_pool,
        tc.tile_pool(name="it", bufs=2) as it_pool,
        tc.tile_pool(name="ps1", bufs=2, space="PSUM") as ps1,
        tc.tile_pool(name="ps2", bufs=6, space="PSUM") as ps2,
    ):
        # Adapter weights, loaded once (contraction dim on partitions), on
        # the scalar engine's ring so the sync ring starts x/base at t=0.
        a_sb = cpool.tile([P, DC2, 2, R], FP8)
        nc.scalar.dma_start(a_sb[:], a_w[:])
        b_sb = cpool.tile([R, D], BF16)
        nc.scalar.dma_start(b_sb[:], b_w[:])

        t0 = 0
        for gi, tg in enumerate(GROUPS):
            # One contiguous load for the whole group's x (fp8).
            xc = xc_pool.tile([P, DC2, 2, tg], FP8)
            nc.sync.dma_start(xc[:], xh[:, t0 * DC : (t0 + tg) * DC])

            # GEMM1: interT[r, t] = 64 * sum_c A_c.T @ x_c, accumulated in
            # PSUM. DoubleRow: each partition carries 2 contraction rows
            # (d = c2*256 + i*128 + p), so 16 matmuls at 2 rows/cycle.
            it_ps = ps1.tile([R, tg], F32)
            for c2 in range(DC2):
                nc.tensor.matmul(
                    it_ps[:],
                    a_sb[:, c2, :, :],
                    xc[:, c2, :, :],
                    start=(c2 == 0),
                    stop=(c2 == DC2 - 1),
                    perf_mode=mybir.MatmulPerfMode.DoubleRow,
                )

            # evacuate to bf16 (GEMM2 stationary operand)
            it_sb = it_pool.tile([R, tg], BF16)
            nc.vector.tensor_copy(it_sb[:], it_ps[:])

            # The last group runs all adds on DVE (lowest-latency path) and
            # stores chunk-by-chunk to drain the kernel fast.
            drain = gi == len(GROUPS) - 1
            for sub in range(tg // P):
                tt = t0 + sub * P
                bs = bs_pool.tile([P, D], BF16)
                nc.sync.dma_start(bs[:], base[tt : tt + P, :])
                for o in range(OC):
                    y_ps = ps2.tile([P, OCH], F32)
                    nc.tensor.matmul(
                        y_ps[:],
                        it_sb[:, sub * P : (sub + 1) * P],
                        b_sb[:, o * OCH : (o + 1) * OCH],
                        start=True,
                        stop=True,
                    )
                    dst = bs[:, o * OCH : (o + 1) * OCH]
                    # Spread the PSUM-evac + base-add work across engines so
                    # PSUM frees faster than PE produces it. GpSimd can't
                    # read PSUM, so odd chunks go PSUM -> ACT copy -> SBUF ->
                    # GpSimd add.
                    if o % 2 == 0 or drain:
                        nc.vector.tensor_add(dst, dst, y_ps[:])
                    else:
                        ye = ye_pool.tile([P, OCH], BF16)
                        nc.scalar.activation(
                            ye[:], y_ps[:], mybir.ActivationFunctionType.Copy
                        )
                        nc.gpsimd.tensor_add(dst, dst, ye[:])
                    if drain and o % 2 == 1:
                        # store pairs of o-chunks as soon as their adds land,
                        # alternating rings to shorten the final flush
                        eng = nc.sync if o % 4 == 1 else nc.scalar
                        eng.dma_start(
                            out[tt : tt + P, (o - 1) * OCH : (o + 1) * OCH],
                            bs[:, (o - 1) * OCH : (o + 1) * OCH],
                        )
                if not drain:
                    nc.scalar.dma_start(out[tt : tt + P, :], bs[:])
            t0 += tg


def shard_inputs(x, base_output, adapter_indices, lora_A, lora_B, lora_scaling):
    idx = np.asarray(adapter_indices).astype(np.int64)
    a_b = np.asarray(lora_A, dtype=np.float32)[idx]        # [B, D, R]
    b_b = np.asarray(lora_B, dtype=np.float32)[idx]        # [B, R, D]
    s_b = np.asarray(lora_scaling, dtype=np.float32)[idx]  # [B]
    # A is scaled x64 into e4m3 normal range; the 1/64 is folded into B.
    b_scaled = (b_b * (s_b[:, None, None] / 64.0)).astype(ml_dtypes.bfloat16)
    # a_w[p, c2, i, r] = 64*A[c2*256+i*128+p, r]
    a_sw = (
        (64.0 * a_b)
        .reshape(B, DC2, 2, P, R)
        .transpose(0, 3, 1, 2, 4)
        .astype(ml_dtypes.float8_e4m3)
    )
    xs = np.asarray(x, dtype=np.float32)
    bs = np.asarray(base_output, dtype=np.float32).astype(ml_dtypes.bfloat16)
    maps = []
    for b in range(B):
        # group-major packing: per group (t0, tg) a [P, DC2, 2, tg] block,
        # xh[p, t0*DC + c2*2*tg + i*tg + t] = x[b, t0+t, c2*256+i*128+p]
        blocks = []
        t0 = 0
        xtb = xs[b].T.reshape(DC2, 2, P, T)  # [c2, i, p, t]
        for tg in GROUPS:
            blocks.append(
                xtb[:, :, :, t0 : t0 + tg].transpose(2, 0, 1, 3).reshape(P, DC * tg)
            )
            t0 += tg
        xt = np.concatenate(blocks, axis=1)
        maps.append(
            {
                "xh": np.ascontiguousarray(xt.astype(ml_dtypes.float8_e4m3)),
                "base": np.ascontiguousarray(bs[b]),
                "a_w": np.ascontiguousarray(a_sw[b]),
                "b_w": np.ascontiguousarray(b_scaled[b]),
            }
        )
    return maps


def run(inputs: dict, trace: bool = False, **kwargs):
    """Build + run on 8 cores. Returns (output [B,T,D] f32, BassKernelResults)."""
    from concourse.bass_utils import run_bass_kernel_spmd

    nc = build_program()
    in_maps = shard_inputs(**inputs)
    res = run_bass_kernel_spmd(
        nc, in_maps, core_ids=list(range(B)), trace=trace, **kwargs
    )
    out = np.stack(
        [np.asarray(res.results[b]["out"]).astype(np.float32) for b in range(B)],
        axis=0,
    )
    return out, res


def kernel(x, base_output, adapter_indices, lora_A, lora_B, lora_scaling):
    out, _ = run(
        dict(
            x=x,
            base_output=base_output,
            adapter_indices=adapter_indices,
            lora_A=lora_A,
            lora_B=lora_B,
            lora_scaling=lora_scaling,
        )
    )
    return out


# revision 28
# speedup vs baseline: 1.0535x; 1.0227x over previous
"""LoRA-with-routing kernel for Trainium2 (8 NeuronCores, SPMD).

out[b] = base[b] + (x[b] @ lora_A[idx[b]]) @ lora_B[idx[b]] * s[idx[b]]

Sharding: data-parallel over batch (B=8 rows, one per core). The adapter
gather (routing) happens host-side while sharding: each core receives its
batch row plus that row's adapter weights.

The kernel is HBM-bandwidth bound (~358 GB/s per core), so HBM traffic is
minimized: x and A are e4m3 fp8 (A scaled x64 into e4m3 normal range, the
1/64 + lora_scaling folded into B), base is bf16, and the output is stored
bf16 and upcast host-side after the gather. Per-core traffic:
8 (x) + 16 (base) + 16 (out) + 0.75 (weights) ~= 41 MiB.

Device pipeline per core (T=2048, D=4096, R=64), per 256-token group:
  1. load x group tile (one fully-contiguous 1 MiB DMA, sync ring)
  2. GEMM1 (PE, DoubleRow fp8 perf mode, 2 contraction rows/cycle):
     interT[64 r, tg] += 64 * A_c.T @ x_c  (accum 16 double-chunks)
  3. DVE evac interT -> bf16 SBUF
  4. per 128-token subtile: load base bf16 (sync), GEMM2 y[128,512] =
     interT.T @ B (bf16), add base: even o-chunks DVE direct from PSUM,
     odd o-chunks ACT-copy to SBUF + GpSimd add (GpSimd can't read PSUM);
     store bf16 rows (scalar ring)
  5. final group drains per 256 KiB chunk pair on alternating rings so the
     kernel tail flushes fast

Engine budget per the traces: DMA ~85-100% active (the wall), PE ~60%,
DVE ~35%, ACT ~35%, GpSimd ~45%. An HBM activity throttle (50%-duty
windows after ~35us) makes run-to-run times vary ~123-139us.
"""

import sys

for _p in ("/opt/trn_rl_repo", "/root/.axon_site/_ro/trn_rl_repo"):
    if _p not in sys.path:
        sys.path.append(_p)

import numpy as np
import ml_dtypes

import concourse.bass as bass
import concourse.bacc as bacc
import concourse.mybir as mybir
from concourse import tile

B, T, D, R = 8, 2048, 4096, 64
P = 128          # partitions
DC = D // P      # 32 d-chunks (contraction)
DC2 = DC // 2    # 16 double-chunks (DoubleRow matmul: 2 k-rows/partition)
# Uniform fine groups keep all DMA streams (x/base/out) flowing together;
# long groups starve the store stream during GEMM1 phases.
GROUPS = [256] * 8
OCH = 512        # output free chunk (one PSUM bank of f32)
OC = D // OCH    # 8 o-chunks

F32 = mybir.dt.float32
BF16 = mybir.dt.bfloat16
# e4m3 for x and A enables the PE DoubleRow perf mode (2 contraction rows
# per cycle, halving GEMM1 time). A is scaled x64 on the host (its values
# are bounded by 1/64 = e4m3's min normal); 1/64 is folded into B.
FP8 = mybir.dt.float8e4


def build_program():
    nc = bacc.Bacc("TRN2", target_bir_lowering=False, debug=False, num_devices=B)
    # x pre-packed host-side, group-major then d-chunk-major per partition:
    # for group (t0, tg), columns [t0*DC : (t0+tg)*DC] hold [DC, tg] blocks
    # with xh[p, t0*DC + c*tg + t] = x[t0+t, c*128+p]
    xh = nc.dram_tensor("xh", [P, DC * T], FP8, kind="ExternalInput").ap()
    base = nc.dram_tensor("base", [T, D], BF16, kind="ExternalInput").ap()
    # A pre-swizzled host-side: a_w[p, c2, i, r] = 64*A[c2*256+i*128+p, r]
    a_w = nc.dram_tensor("a_w", [P, DC2, 2, R], FP8, kind="ExternalInput").ap()
    b_w = nc.dram_tensor("b_w", [R, D], BF16, kind="ExternalInput").ap()
    out = nc.dram_tensor("out", [T, D], BF16, kind="ExternalOutput").ap()

    with tile.TileContext(nc) as tc:
        _body(tc, xh, base, a_w, b_w, out)
    nc.compile()
    return nc


def _body(tc, xh, base, a_w, b_w, out):
    nc = tc.nc
    with (
        tc.tile_pool(name="const", bufs=1) as cpool,
        tc.tile_pool(name="xc", bufs=4) as xc_pool,
        tc.tile_pool(name="bs", bufs=8) as bs_pool,
        tc.tile_pool(name="ye", bufs=4) as ye_pool,
        tc.tile_pool(name="it", bufs=2) as it_pool,
        tc.tile_pool(name="ps1", bufs=2, space="PSUM") as ps1,
        tc.tile_pool(name="ps2", bufs=6, space="PSUM") as ps2,
    ):
        # Adapter weights, loaded once (contraction dim on partitions), on
        # the scalar engine's ring so the sync ring starts x/base at t=0.
        a_sb = cpool.tile([P, DC2, 2, R], FP8)
        nc.scalar.dma_start(a_sb[:], a_w[:])
        b_sb = cpool.tile([R, D], BF16)
        nc.scalar.dma_start(b_sb[:], b_w[:])

        t0 = 0
        for gi, tg in enumerate(GROUPS):
            # One contiguous load for the whole group's x (fp8).
            xc = xc_pool.tile([P, DC2, 2, tg], FP8)
            nc.sync.dma_start(xc[:], xh[:, t0 * DC : (t0 + tg) * DC])

            # GEMM1: interT[r, t] = 64 * sum_c A_c.T @ x_c, accumulated in
            # PSUM. DoubleRow: each partition carries 2 contraction rows
            # (d = c2*256 + i*128 + p), so 16 matmuls at 2 rows/cycle.
            it_ps = ps1.tile([R, tg], F32)
            for c2 in range(DC2):
                nc.tensor.matmul(
                    it_ps[:],
                    a_sb[:, c2, :, :],
                    xc[:, c2, :, :],
                    start=(c2 == 0),
                    stop=(c2 == DC2 - 1),
                    perf_mode=mybir.MatmulPerfMode.DoubleRow,
                )

            # evacuate to bf16 (GEMM2 stationary operand)
            it_sb = it_pool.tile([R, tg], BF16)
            nc.vector.tensor_copy(it_sb[:], it_ps[:])

            # The last group runs all adds on DVE (lowest-latency path) and
            # stores chunk-by-chunk to drain the kernel fast.
            drain = gi == len(GROUPS) - 1
            for sub in range(tg // P):
                tt = t0 + sub * P
                bs = bs_pool.tile([P, D], BF16)
                nc.sync.dma_start(bs[:], base[tt : tt + P, :])
                for o in range(OC):
                    y_ps = ps2.tile([P, OCH], F32)
                    nc.tensor.matmul(
                        y_ps[:],
                        it_sb[:, sub * P : (sub + 1) * P],
                        b_sb[:, o * OCH : (o + 1) * OCH],
                        start=True,
                        stop=True,
                    )
                    dst = bs[:, o * OCH : (o + 1) * OCH]
                    # Spread the PSUM-evac + base-add work across engines so
                    # PSUM frees faster than PE produces it. GpSimd can't
                    # read PSUM, so odd chunks go PSUM -> ACT copy -> SBUF ->
                    # GpSimd add.
                    if o % 2 == 0 or drain:
                        nc.vector.tensor_add(dst, dst, y_ps[:])
                    else:
                        ye = ye_pool.tile([P, OCH], BF16)
                        nc.scalar.activation(
                            ye[:], y_ps[:], mybir.ActivationFunctionType.Copy
                        )
                        nc.gpsimd.tensor_add(dst, dst, ye[:])
                    if drain and o % 2 == 1:
                        # store pairs of o-chunks as soon as their adds land,
                        # alternating rings to shorten the final flush
                        eng = nc.sync if o % 4 == 1 else nc.scalar
                        eng.dma_start(
                            out[tt : tt + P, (o - 1) * OCH : (o + 1) * OCH],
                            bs[:, (o - 1) * OCH : (o + 1) * OCH],
                        )
                if not drain:
                    nc.scalar.dma_start(out[tt : tt + P, :], bs[:])
            t0 += tg


def shard_inputs(x, base_output, adapter_indices, lora_A, lora_B, lora_scaling):
    idx = np.asarray(adapter_indices).astype(np.int64)
    a_b = np.asarray(lora_A, dtype=np.float32)[idx]        # [B, D, R]
    b_b = np.asarray(lora_B, dtype=np.float32)[idx]        # [B, R, D]
    s_b = np.asarray(lora_scaling, dtype=np.float32)[idx]  # [B]
    # A is scaled x64 into e4m3 normal range; the 1/64 is folded into B.
    b_scaled = (b_b * (s_b[:, None, None] / 64.0)).astype(ml_dtypes.bfloat16)
    # a_w[p, c2, i, r] = 64*A[c2*256+i*128+p, r]
    a_sw = (
        (64.0 * a_b)
        .reshape(B, DC2, 2, P, R)
        .transpose(0, 3, 1, 2, 4)
        .astype(ml_dtypes.float8_e4m3)
    )
    xs = np.asarray(x, dtype=np.float32)
    bs = np.asarray(base_output, dtype=np.float32).astype(ml_dtypes.bfloat16)
    maps = []
    for b in range(B):
        # group-major packing: per group (t0, tg) a [P, DC2, 2, tg] block,
        # xh[p, t0*DC + c2*2*tg + i*tg + t] = x[b, t0+t, c2*256+i*128+p]
        blocks = []
        t0 = 0
        xtb = xs[b].T.reshape(DC2, 2, P, T)  # [c2, i, p, t]
        for tg in GROUPS:
            blocks.append(
                xtb[:, :, :, t0 : t0 + tg].transpose(2, 0, 1, 3).reshape(P, DC * tg)
            )
            t0 += tg
        xt = np.concatenate(blocks, axis=1)
        maps.append(
            {
                "xh": np.ascontiguousarray(xt.astype(ml_dtypes.float8_e4m3)),
                "base": np.ascontiguousarray(bs[b]),
                "a_w": np.ascontiguousarray(a_sw[b]),
                "b_w": np.ascontiguousarray(b_scaled[b]),
            }
        )
    return maps


def run(inputs: dict, trace: bool = False, **kwargs):
    """Build + run on 8 cores. Returns (output [B,T,D] f32, BassKernelResults)."""
    from concourse.bass_utils import run_bass_kernel_spmd

    nc = build_program()
    in_maps = shard_inputs(**inputs)
    res = run_bass_kernel_spmd(
        nc, in_maps, core_ids=list(range(B)), trace=trace, **kwargs
    )
    out = np.stack(
        [np.asarray(res.results[b]["out"]).astype(np.float32) for b in range(B)],
        axis=0,
    )
    return out, res


def kernel(x, base_output, adapter_indices, lora_A, lora_B, lora_scaling):
    out, _ = run(
        dict(
            x=x,
            base_output=base_output,
            adapter_indices=adapter_indices,
            lora_A=lora_A,
            lora_B=lora_B,
            lora_scaling=lora_scaling,
        )
    )
    return out


# revision 29
# speedup vs baseline: 1.1905x; 1.1300x over previous
"""LoRA-with-routing kernel for Trainium2 (8 NeuronCores, SPMD).

out[b] = base[b] + (x[b] @ lora_A[idx[b]]) @ lora_B[idx[b]] * s[idx[b]]

Sharding: data-parallel over batch (B=8 rows, one per core). The adapter
gather (routing) happens host-side while sharding: each core receives its
batch row plus that row's adapter weights.

The kernel is HBM-bandwidth bound (~358 GB/s per core), so HBM traffic is
minimized: x and A are e4m3 fp8 (A scaled x64 into e4m3 normal range, the
1/64 + lora_scaling folded into B), base is bf16, and the output is stored
bf16 and upcast host-side after the gather. Per-core traffic:
8 (x) + 16 (base) + 16 (out) + 0.75 (weights) ~= 41 MiB.

Device pipeline per core (T=2048, D=4096, R=64), per 256-token group:
  1. load x group tile (one fully-contiguous 1 MiB DMA, sync ring)
  2. GEMM1 (PE, DoubleRow fp8 perf mode, 2 contraction rows/cycle):
     interT[64 r, tg] += 64 * A_c.T @ x_c  (accum 16 double-chunks)
  3. DVE evac interT -> bf16 SBUF
  4. per 128-token subtile: load base bf16 (sync), GEMM2 y[128,512] =
     interT.T @ B (bf16), add base: even o-chunks DVE direct from PSUM,
     odd o-chunks ACT-copy to SBUF + GpSimd add (GpSimd can't read PSUM);
     store bf16 rows (scalar ring)
  5. final group drains per 256 KiB chunk pair on alternating rings so the
     kernel tail flushes fast

Engine budget per the traces: DMA ~85-100% active (the wall), PE ~60%,
DVE ~35%, ACT ~35%, GpSimd ~45%. An HBM activity throttle (50%-duty
windows after ~35us) makes run-to-run times vary ~123-139us.
"""

import sys

for _p in ("/opt/trn_rl_repo", "/root/.axon_site/_ro/trn_rl_repo"):
    if _p not in sys.path:
        sys.path.append(_p)

import numpy as np
import ml_dtypes

import concourse.bass as bass
import concourse.bacc as bacc
import concourse.mybir as mybir
from concourse import tile

B, T, D, R = 8, 2048, 4096, 64
P = 128          # partitions
DC = D // P      # 32 d-chunks (contraction)
DC2 = DC // 2    # 16 double-chunks (DoubleRow matmul: 2 k-rows/partition)
# Uniform fine groups keep all DMA streams (x/base/out) flowing together;
# long groups starve the store stream during GEMM1 phases.
GROUPS = [256] * 8
OCH = 512        # output free chunk (one PSUM bank of f32)
OC = D // OCH    # 8 o-chunks

F32 = mybir.dt.float32
BF16 = mybir.dt.bfloat16
# e4m3 for x and A enables the PE DoubleRow perf mode (2 contraction rows
# per cycle, halving GEMM1 time). A is scaled x64 on the host (its values
# are bounded by 1/64 = e4m3's min normal); 1/64 is folded into B.
FP8 = mybir.dt.float8e4


def build_program():
    nc = bacc.Bacc("TRN2", target_bir_lowering=False, debug=False, num_devices=B)
    # x pre-packed host-side, group-major then d-chunk-major per partition:
    # for group (t0, tg), columns [t0*DC : (t0+tg)*DC] hold [DC, tg] blocks
    # with xh[p, t0*DC + c*tg + t] = x[t0+t, c*128+p]
    xh = nc.dram_tensor("xh", [P, DC * T], FP8, kind="ExternalInput").ap()
    base = nc.dram_tensor("base", [T, D], BF16, kind="ExternalInput").ap()
    # A pre-swizzled host-side: a_w[p, c2, i, r] = 64*A[c2*256+i*128+p, r]
    a_w = nc.dram_tensor("a_w", [P, DC2, 2, R], FP8, kind="ExternalInput").ap()
    b_w = nc.dram_tensor("b_w", [R, D], BF16, kind="ExternalInput").ap()
    out = nc.dram_tensor("out", [T, D], BF16, kind="ExternalOutput").ap()

    with tile.TileContext(nc) as tc:
        _body(tc, xh, base, a_w, b_w, out)
    nc.compile()
    return nc


def _body(tc, xh, base, a_w, b_w, out):
    nc = tc.nc
    with (
        tc.tile_pool(name="const", bufs=1) as cpool,
        tc.tile_pool(name="xc", bufs=4) as xc_pool,
        tc.tile_pool(name="bs", bufs=8) as bs_pool,
        tc.tile_pool(name="ye", bufs=4) as ye_pool,
        tc.tile_pool(name="it", bufs=2) as it_pool,
        tc.tile_pool(name="ps1", bufs=2, space="PSUM") as ps1,
        tc.tile_pool(name="ps2", bufs=6, space="PSUM") as ps2,
    ):
        # Adapter weights, loaded once (contraction dim on partitions), on
        # the scalar engine's ring so the sync ring starts x/base at t=0.
        a_sb = cpool.tile([P, DC2, 2, R], FP8)
        nc.scalar.dma_start(a_sb[:], a_w[:])
        b_sb = cpool.tile([R, D], BF16)
        nc.scalar.dma_start(b_sb[:], b_w[:])

        t0 = 0
        for gi, tg in enumerate(GROUPS):
            # One contiguous load for the whole group's x (fp8).
            xc = xc_pool.tile([P, DC2, 2, tg], FP8)
            nc.sync.dma_start(xc[:], xh[:, t0 * DC : (t0 + tg) * DC])

            # GEMM1: interT[r, t] = 64 * sum_c A_c.T @ x_c, accumulated in
            # PSUM. DoubleRow: each partition carries 2 contraction rows
            # (d = c2*256 + i*128 + p), so 16 matmuls at 2 rows/cycle.
            it_ps = ps1.tile([R, tg], F32)
            for c2 in range(DC2):
                nc.tensor.matmul(
                    it_ps[:],
                    a_sb[:, c2, :, :],
                    xc[:, c2, :, :],
                    start=(c2 == 0),
                    stop=(c2 == DC2 - 1),
                    perf_mode=mybir.MatmulPerfMode.DoubleRow,
                )

            # evacuate to bf16 (GEMM2 stationary operand)
            it_sb = it_pool.tile([R, tg], BF16)
            nc.vector.tensor_copy(it_sb[:], it_ps[:])

            # The last group runs all adds on DVE (lowest-latency path) and
            # stores chunk-by-chunk to drain the kernel fast.
            drain = gi == len(GROUPS) - 1
            for sub in range(tg // P):
                tt = t0 + sub * P
                bs = bs_pool.tile([P, D], BF16)
                nc.sync.dma_start(bs[:], base[tt : tt + P, :])
                for o in range(OC):
                    y_ps = ps2.tile([P, OCH], F32)
                    nc.tensor.matmul(
                        y_ps[:],
                        it_sb[:, sub * P : (sub + 1) * P],
                        b_sb[:, o * OCH : (o + 1) * OCH],
                        start=True,
                        stop=True,
                    )
                    dst = bs[:, o * OCH : (o + 1) * OCH]
                    # Spread the PSUM-evac + base-add work across engines so
                    # PSUM frees faster than PE produces it. GpSimd can't
                    # read PSUM, so its chunks go PSUM -> ACT copy -> SBUF ->
                    # GpSimd add. 5:3 split matches DVE's 686ns/add vs
                    # GpSimd's 1150ns so both finish together.
                    if o in (0, 2, 4, 5, 6):
                        nc.vector.tensor_add(dst, dst, y_ps[:])
                    else:
                        ye = ye_pool.tile([P, OCH], BF16)
                        nc.scalar.activation(
                            ye[:], y_ps[:], mybir.ActivationFunctionType.Copy
                        )
                        nc.gpsimd.tensor_add(dst, dst, ye[:])
                    if drain and o % 2 == 1:
                        # store pairs of o-chunks as soon as their adds land,
                        # alternating rings to shorten the final flush
                        eng = nc.sync if o % 4 == 1 else nc.scalar
                        eng.dma_start(
                            out[tt : tt + P, (o - 1) * OCH : (o + 1) * OCH],
                            bs[:, (o - 1) * OCH : (o + 1) * OCH],
                        )
                if not drain:
                    nc.scalar.dma_start(out[tt : tt + P, :], bs[:])
            t0 += tg


def shard_inputs(x, base_output, adapter_indices, lora_A, lora_B, lora_scaling):
    idx = np.asarray(adapter_indices).astype(np.int64)
    a_b = np.asarray(lora_A, dtype=np.float32)[idx]        # [B, D, R]
    b_b = np.asarray(lora_B, dtype=np.float32)[idx]        # [B, R, D]
    s_b = np.asarray(lora_scaling, dtype=np.float32)[idx]  # [B]
    # A is scaled x64 into e4m3 normal range; the 1/64 is folded into B.
    b_scaled = (b_b * (s_b[:, None, None] / 64.0)).astype(ml_dtypes.bfloat16)
    # a_w[p, c2, i, r] = 64*A[c2*256+i*128+p, r]
    a_sw = (
        (64.0 * a_b)
        .reshape(B, DC2, 2, P, R)
        .transpose(0, 3, 1, 2, 4)
        .astype(ml_dtypes.float8_e4m3)
    )
    xs = np.asarray(x, dtype=np.float32)
    bs = np.asarray(base_output, dtype=np.float32).astype(ml_dtypes.bfloat16)
    maps = []
    for b in range(B):
        # group-major packing: per group (t0, tg) a [P, DC2, 2, tg] block,
        # xh[p, t0*DC + c2*2*tg + i*tg + t] = x[b, t0+t, c2*256+i*128+p]
        blocks = []
        t0 = 0
        xtb = xs[b].T.reshape(DC2, 2, P, T)  # [c2, i, p, t]
        for tg in GROUPS:
            blocks.append(
                xtb[:, :, :, t0 : t0 + tg].transpose(2, 0, 1, 3).reshape(P, DC * tg)
            )
            t0 += tg
        xt = np.concatenate(blocks, axis=1)
        maps.append(
            {
                "xh": np.ascontiguousarray(xt.astype(ml_dtypes.float8_e4m3)),
                "base": np.ascontiguousarray(bs[b]),
                "a_w": np.ascontiguousarray(a_sw[b]),
                "b_w": np.ascontiguousarray(b_scaled[b]),
            }
        )
    return maps


def run(inputs: dict, trace: bool = False, **kwargs):
    """Build + run on 8 cores. Returns (output [B,T,D] f32, BassKernelResults)."""
    from concourse.bass_utils import run_bass_kernel_spmd

    nc = build_program()
    in_maps = shard_inputs(**inputs)
    res = run_bass_kernel_spmd(
        nc, in_maps, core_ids=list(range(B)), trace=trace, **kwargs
    )
    out = np.stack(
        [np.asarray(res.results[b]["out"]).astype(np.float32) for b in range(B)],
        axis=0,
    )
    return out, res


def kernel(x, base_output, adapter_indices, lora_A, lora_B, lora_scaling):
    out, _ = run(
        dict(
            x=x,
            base_output=base_output,
            adapter_indices=adapter_indices,
            lora_A=lora_A,
            lora_B=lora_B,
            lora_scaling=lora_scaling,
        )
    )
    return out
